# revision 26
# baseline (speedup 1.0000x reference)
"""ArcFace-style sub-center loss (topk_masking) on 8 Trainium2 NeuronCores.

v4 strategy (class-parallel, 752 classes/core, pure-matmul device kernel):
  - Host pre-normalizes x and w rows (0.07% of model FLOPs), transposes
    both, and ships f16: xnT [512,1024], wnT [3,512,752] per core. The
    device does no norms and no input transposes; DMA is 3.3MB/core and
    the first cosine matmul issues at ~6us.
  - Per batch tile (128 rows): 24 f16 matmuls (k-major, 6 consecutive
    MMs share the stationary xnT block), psum chunks (512|240) merged
    over the 3 sub-centers by ACT copy + 2 DVE maxes into a contiguous
    [128,752] f32 cosine slab. Max8 writes the AG payload top-8 lanes
    directly; the label cosine is gathered by an (iota==label)*slab
    row-reduce; one ACT Exp pass with accum produces S_loc =
    sum(exp(30*cos)) in RAW exp space (args <= ~11, f32-safe), so no
    row-max bias pass and no AllReduce are needed anywhere.
  - ONE AllGather total ([1024,10] payload = top8 | cosl | S_loc).
    Collectives serialize on the gpsimd queue with ~10us dispatch + ~8us
    exec each, so fewer is strictly better; the CC ring arming happens
    during the matmul phase.
  - The AG result transpose ([8 ranks,1024,10] -> row-major) is done as
    ONE line-rate DMA into a [64,1280] staging tile plus 10 PE
    transposes, instead of ~8k 40-byte DMA descriptors.
  - Margin fixups in raw exp space; sine via Taylor 1 - c^2/2 - c^4/8
    (cosines here are < 0.4); ACT tables load exactly twice (Exp, Ln).
  - loss_row = ln(S + corr) - 30*phi_l; loss/prec cross-partition
    reduced by a ones-matmul; core 0 returns the [1,2] result.
"""

import math

import numpy as np

import concourse.bass as bass
import concourse.mybir as mybir
import concourse.tile as tile
from concourse import bacc
from concourse.bass import ds, ts
from concourse.bass_utils import run_bass_kernel_spmd
from concourse.masks import make_identity

F32 = mybir.dt.float32
F16 = mybir.dt.float16
I32 = mybir.dt.int32
AOP = mybir.AluOpType
AF = mybir.ActivationFunctionType
AX = mybir.AxisListType

B, NOUT, NCLASSES, CENTER, TOPK = 1024, 512, 5994, 3, 5
NCORES = 8
CPCW = 752                    # classes per core (core 7: 730 real + 22 pad)
NPAD = float(NCORES * CPCW - NCLASSES)  # 22 zero-weight pad columns
NBT = B // 128                # 8 batch tiles
KT = NOUT // 128              # 4 contraction chunks
N0, N1 = 512, CPCW - 512      # psum chunk widths (bank-aligned)
SCALE = 30.0
AGW = 10                      # AG payload floats/row: top8 | cosl | S_loc

M, SUB_M = 0.2, -0.06
COS_M, SIN_M = math.cos(M), math.sin(M)
SUB_COS_M, SUB_SIN_M = math.cos(SUB_M), math.sin(SUB_M)

_CACHE = {}


def _build():
    nc = bacc.Bacc("TRN2", target_bir_lowering=False, debug=False,
                   num_devices=NCORES)
    x_d = nc.dram_tensor("xnT", [NOUT, B], F16, kind="ExternalInput")
    w_d = nc.dram_tensor("wnT", [CENTER, NOUT, CPCW], F16,
                         kind="ExternalInput")
    lab_d = nc.dram_tensor("labels", [128, NBT], F32, kind="ExternalInput")
    out_d = nc.dram_tensor("out", [1, 2], F32, kind="ExternalOutput")

    with tile.TileContext(nc) as tc:
        with (
            tc.tile_pool(name="const", bufs=1) as constp,
            tc.tile_pool(name="big", bufs=1) as bigp,
            tc.tile_pool(name="slab", bufs=4) as slabp,
            tc.tile_pool(name="scr", bufs=2) as scrp,
            tc.tile_pool(name="gscr", bufs=2) as gscrp,
            tc.tile_pool(name="small", bufs=1) as smallp,
            tc.tile_pool(name="pay", bufs=NBT) as payp,
            tc.tile_pool(name="psA", bufs=4, space="PSUM") as psA,
            tc.tile_pool(name="dram", bufs=1, space="DRAM") as dramp,
        ):
            # ---- constants (gpsimd queue: consts, then ONLY the AG) ----
            iota_i = constp.tile([128, CPCW], I32, tag="iotai")
            nc.gpsimd.iota(iota_i[:], pattern=[[1, CPCW]], base=0,
                           channel_multiplier=0)
            identity = constp.tile([128, 128], F32, tag="ident")
            make_identity(nc, identity[:])
            ones = constp.tile([128, 1], F32, tag="ones")
            nc.vector.memset(ones[:], 1.0)
            iota_f = constp.tile([128, CPCW], F32, tag="iotaf")
            nc.vector.tensor_copy(iota_f[:], iota_i[:])
            labs = constp.tile([128, NBT], F32, tag="labs")
            nc.sync.dma_start(labs[:], lab_d[:])


            # ---- inputs: already normalized + transposed on host ----
            xnT = bigp.tile([128, KT, B], F16, tag="xnT")
            wnT = bigp.tile([128, CENTER, KT, CPCW], F16, tag="wnT")
            for k in range(KT):
                for a in range(CENTER):
                    nc.sync.dma_start(wnT[:, a, k, :],
                                      w_d[a, ds(k * 128, 128), :])
                nc.sync.dma_start(xnT[:, k, :], x_d[ds(k * 128, 128), :])

            # ---- per-batch-tile: cosine slab, top8, label gather, exp ----
            pays = [payp.tile([128, AGW], F32, tag="pay", name=f"pay{t}")
                    for t in range(NBT)]
            ag_ins = [dramp.tile([B // 2, AGW], F32, tag=f"agin{h}",
                                 name=f"agin{h}") for h in range(2)]

            # ---- two AllGathers: [bt 0-3] and [bt 4-7] ----
            # AG1's dispatch gap + mesh handshake overlap the matmul phase;
            # AG2 (second collective) begins ~1us after its trigger. Half-0
            # gather/merge/fixups run during AG2's flight.
            HB = NBT // 2
            ag_outs = [dramp.tile([NCORES, HB * 128, AGW], F32,
                                  tag=f"agout{h}", name=f"agout{h}")
                       for h in range(2)]
            stages = [smallp.tile([NCORES * HB, 128 * AGW], F32,
                                  tag=f"stage{h}", name=f"stage{h}")
                      for h in range(2)]
            gallTs = [smallp.tile([128, AGW * NCORES * HB], F32,
                                  tag=f"gallT{h}", name=f"gallT{h}")
                      for h in range(2)]
            g8 = smallp.tile([128, NBT * 8], F32, tag="g8")
            g3 = g8[:].rearrange("p (t k) -> p t k", k=8)
            CS = smallp.tile([128, 2 * NBT], F32, tag="CS")
            CS3 = CS[:].rearrange("p (j t) -> p j t", j=2)
            cosl = CS3[:, 0, :]
            Sg = CS3[:, 1, :]
            SC = smallp.tile([128, NBT], F32, tag="SC")

            def emit_ag(h):
                nc.gpsimd.collective_compute(
                    "AllGather", AOP.bypass,
                    replica_groups=[list(range(NCORES))],
                    ins=[ag_ins[h][:].opt()],
                    outs=[ag_outs[h][:].opt()])



            for bt in range(NBT):
                slab = slabp.tile([128, CPCW], F32, tag="slab")
                pas = [psA.tile([128, 2, N0], F32, tag="psA",
                                name=f"psA_{bt}_{a}") for a in range(CENTER)]
                for k in range(KT):
                    lhs = xnT[:, k, ts(bt, 128)]
                    for a in range(CENTER):
                        nc.tensor.matmul(pas[a][:, 0, :], lhs,
                                         wnT[:, a, k, 0:N0],
                                         start=(k == 0), stop=(k == KT - 1))
                        nc.tensor.matmul(pas[a][:, 1, 0:N1], lhs,
                                         wnT[:, a, k, N0:CPCW],
                                         start=(k == 0), stop=(k == KT - 1))
                pavs = [p[:].rearrange("p a b -> p (a b)")[:, 0:CPCW]
                        for p in pas]
                nc.scalar.copy(slab[:], pavs[0])
                nc.vector.tensor_tensor(slab[:], pavs[1], slab[:], op=AOP.max)
                nc.vector.tensor_tensor(slab[:], pavs[2], slab[:], op=AOP.max)
                nc.vector.max(pays[bt][:, 0:8], slab[:])
                gscr = gscrp.tile([128, CPCW], F32, tag="gscr")
                nc.vector.scalar_tensor_tensor(
                    out=gscr[:], in0=iota_f[:], scalar=labs[:, ds(bt, 1)],
                    in1=slab[:], op0=AOP.is_equal, op1=AOP.mult,
                    accum_out=pays[bt][:, 8:9])
                escr = scrp.tile([128, CPCW], F32, tag="scr750")
                nc.scalar.activation(escr[:], slab[:], AF.Exp, scale=SCALE,
                                     accum_out=pays[bt][:, 9:10])
                nc.sync.dma_start(
                    ag_ins[bt // HB][ts(bt % HB, 128), :], pays[bt][:])
                if bt == HB - 1:
                    emit_ag(0)
                elif bt == NBT - 1:
                    emit_ag(1)

            def emit_gather(h):
                agv = ag_outs[h][:].rearrange("c (t p) j -> (c t) (p j)",
                                              p=128)
                nc.sync.dma_start(stages[h][:], agv)
                gallT4 = gallTs[h][:].rearrange("p (j c t) -> p j c t",
                                                c=NCORES, t=HB)
                stg3 = stages[h][:].rearrange("q (p j) -> q p j", j=AGW)
                NQ = NCORES * HB
                for jj in range(AGW // 2):
                    pg = psA.tile([128, 2, N0], F32, tag="psA",
                                  name=f"psG{h}_{jj}")
                    for j2 in range(2):
                        j = 2 * jj + j2
                        nc.tensor.transpose(pg[:, 0, ds(j2 * NQ, NQ)],
                                            stg3[:, :, j],
                                            identity[0:NQ, 0:NQ])
                    nc.vector.tensor_copy(
                        gallTs[h][:, ds(jj * 2 * NQ, 2 * NQ)],
                        pg[:, 0, 0:2 * NQ])
                for t in range(HB):
                    bt = h * HB + t
                    nc.vector.max(g8[:, ts(bt, 8)], gallT4[:, 0:8, :, t])
                    nc.vector.tensor_reduce(CS3[:, :, bt],
                                            gallT4[:, 8:10, :, t],
                                            axis=AX.X, op=AOP.add)

            def t64(tag, half, w=8):
                return smallp.tile([128, HB * w], F32, tag=f"{tag}{half}",
                                   name=f"{tag}{half}")

            PHILS = []

            def emit_fix(half):
                th = ds(half * HB, HB)
                # A lanes per bt: 0:7 sub_phi args (6 top + label), 7 phi_l,
                # 8:14 raw top-6, 14 raw label cosine, 15 zero.
                A = t64("fxA", half, 16)
                E = t64("fxE", half, 16)
                S2 = t64("fxS2", half)
                Q4 = t64("fxQ4", half)
                SN = t64("fxSN", half)
                isin = t64("fxisin", half, 1)
                A4 = A[:].rearrange("p (t k) -> p t k", k=16)
                E4 = E[:].rearrange("p (t k) -> p t k", k=16)
                S23 = S2[:].rearrange("p (t k) -> p t k", k=8)[:, :, 0:7]
                Q43 = Q4[:].rearrange("p (t k) -> p t k", k=8)[:, :, 0:7]
                SN3 = SN[:].rearrange("p (t k) -> p t k", k=8)
                sn = SN3[:, :, 0:7]
                V = A4[:, :, 8:15]
                PHIL = A4[:, :, 7]
                PHILS.append(PHIL)
                nc.vector.memset(A4[:, :, 15], 0.0)
                nc.vector.tensor_copy(A4[:, :, 8:14], g3[:, th, 0:6])
                nc.vector.tensor_copy(A4[:, :, 14], cosl[:, th])
                # sine = 1 - c^2/2 - c^4/8  (|c| < 0.4 here)
                nc.vector.tensor_tensor(S23, V, V, op=AOP.mult)
                nc.vector.tensor_tensor(Q43, S23, S23, op=AOP.mult)
                nc.vector.tensor_scalar(sn, S23, -0.5, 1.0,
                                        op0=AOP.mult, op1=AOP.add)
                nc.vector.scalar_tensor_tensor(sn, Q43, -0.125, sn,
                                               op0=AOP.mult, op1=AOP.add)
                snl_m = SN3[:, :, 7]
                nc.vector.tensor_scalar_mul(snl_m, SN3[:, :, 6], SIN_M)
                nc.vector.scalar_tensor_tensor(PHIL, A4[:, :, 14], COS_M,
                                               snl_m,
                                               op0=AOP.mult, op1=AOP.subtract)
                nc.vector.tensor_scalar_mul(sn, sn, -SUB_SIN_M)
                nc.vector.scalar_tensor_tensor(A4[:, :, 0:7], V, SUB_COS_M,
                                               sn, op0=AOP.mult, op1=AOP.add)
                nc.scalar.activation(E[:], A[:], AF.Exp, scale=SCALE)
                nc.vector.tensor_tensor(E4[:, :, 0:7], E4[:, :, 0:7],
                                        E4[:, :, 8:15], op=AOP.subtract)
                isv = isin[:].rearrange("p (t k) -> p t k", k=1)
                nc.vector.tensor_tensor(isv[:, :, 0], A4[:, :, 14],
                                        g3[:, th, 5], op=AOP.is_ge)
                nc.vector.tensor_tensor(E4[:, :, 5:7], E4[:, :, 5:7],
                                        isv.to_broadcast([128, HB, 2]),
                                        op=AOP.mult)
                sumF = Q4[:].rearrange("p (t k) -> p t k", k=8)[:, :, 7]
                nc.vector.tensor_reduce(sumF, E4[:, :, 0:6], axis=AX.X,
                                        op=AOP.add)
                nc.vector.tensor_tensor(sumF, sumF, E4[:, :, 6],
                                        op=AOP.subtract)
                nc.vector.tensor_tensor(sumF, sumF, E4[:, :, 7], op=AOP.add)
                nc.vector.tensor_tensor(sumF, sumF, E4[:, :, 14],
                                        op=AOP.subtract)
                nc.vector.scalar_tensor_tensor(SC[:, th], sumF, -NPAD,
                                               Sg[:, th],
                                               op0=AOP.add, op1=AOP.add)

            emit_gather(0)
            emit_fix(0)
            emit_gather(1)
            emit_fix(1)

            # ---- loss / prec reduction ----
            # ln(SC) without the Ln table (the Exp table stays resident):
            # y0 from the f32 exponent bits + linear mantissa, then two
            # Newton steps y += SC*e^-y - 1.
            LN2 = math.log(2.0)
            lnS = smallp.tile([128, NBT], F32, tag="lnS")
            mant = smallp.tile([128, NBT], F32, tag="mant")
            ef = smallp.tile([128, NBT], F32, tag="ef")
            en = smallp.tile([128, NBT], F32, tag="en")
            v = smallp.tile([128, NBT], F32, tag="v")
            stacked = smallp.tile([128, 2], F32, tag="stacked")
            SCi = SC[:].bitcast(I32)
            nc.vector.tensor_scalar(mant[:].bitcast(I32), SCi,
                                    0x7FFFFF, 0x3F800000,
                                    op0=AOP.bitwise_and, op1=AOP.bitwise_or)
            nc.vector.tensor_scalar(ef[:].bitcast(I32), SCi, 23, None,
                                    op0=AOP.logical_shift_right)
            nc.vector.tensor_copy(ef[:], ef[:].bitcast(I32))
            # y0 = ef*ln2 + [(m-1)*0.7071 - 127*ln2]
            nc.vector.tensor_scalar(mant[:], mant[:], 0.7071,
                                    -0.7071 - 127.0 * LN2,
                                    op0=AOP.mult, op1=AOP.add)
            nc.vector.scalar_tensor_tensor(lnS[:], ef[:], LN2, mant[:],
                                           op0=AOP.mult, op1=AOP.add)
            for _ in range(2):
                nc.scalar.activation(en[:], lnS[:], AF.Exp, scale=-1.0)
                nc.vector.tensor_tensor(en[:], en[:], SC[:], op=AOP.mult)
                nc.vector.scalar_tensor_tensor(lnS[:], lnS[:], -1.0, en[:],
                                               op0=AOP.add, op1=AOP.add)
            # loss_row = (lnS - 30*phi_l)/B ; prec_row = (cosl>=t1)*100/B
            for h in range(2):
                th = ds(h * (NBT // 2), NBT // 2)
                nc.vector.scalar_tensor_tensor(lnS[:, th], PHILS[h], -SCALE,
                                               lnS[:, th],
                                               op0=AOP.mult, op1=AOP.add)
            nc.vector.tensor_scalar(mant[:], lnS[:], 1.0 / B, 0.0,
                                    op0=AOP.mult, op1=AOP.add,
                                    accum_out=stacked[:, 0:1])
            nc.vector.tensor_tensor(v[:], cosl, g3[:, :, 0], op=AOP.is_ge)
            nc.vector.tensor_scalar(ef[:], v[:], 100.0 / B, 0.0,
                                    op0=AOP.mult, op1=AOP.add,
                                    accum_out=stacked[:, 1:2])
            fin = psA.tile([128, 2, N0], F32, tag="psA", name="fin")
            nc.tensor.matmul(fin[0:1, 0, 0:2], ones[:], stacked[:],
                             start=True, stop=True)
            res = smallp.tile([128, 2], F32, tag="res")
            nc.vector.tensor_copy(res[0:1, :], fin[0:1, 0, 0:2])
            nc.sync.dma_start(out_d[:], res[0:1, :])

    nc.compile()
    return nc


def _in_maps(x, weight, label):
    x32 = np.asarray(x, dtype=np.float32)
    xn = x32 / np.maximum(np.linalg.norm(x32, axis=1, keepdims=True), 1e-12)
    xnT = np.ascontiguousarray(xn.T).astype(np.float16)   # [512, 1024]

    w32 = np.asarray(weight, dtype=np.float32)
    wn = w32 / np.maximum(np.linalg.norm(w32, axis=2, keepdims=True), 1e-12)
    wpad = np.zeros((CENTER, NCORES * CPCW, NOUT), np.float32)
    wpad[:, :NCLASSES] = wn
    lab = np.asarray(label).astype(np.int64)

    in_maps = []
    for m in range(NCORES):
        wslab = np.ascontiguousarray(
            wpad[:, m * CPCW:(m + 1) * CPCW].transpose(0, 2, 1)
        ).astype(np.float16)                               # [3, 512, 752]
        loc = lab - m * CPCW
        loc = np.where((loc >= 0) & (loc < CPCW), loc, -1)
        labs = np.ascontiguousarray(
            loc.reshape(NBT, 128).T.astype(np.float32))
        in_maps.append({"xnT": xnT, "wnT": wslab, "labels": labs})
    return in_maps


def kernel(x, weight, label):
    if "nc" not in _CACHE:
        _CACHE["nc"] = _build()
    nc = _CACHE["nc"]
    in_maps = _in_maps(x, weight, label)
    res = run_bass_kernel_spmd(nc, in_maps, core_ids=list(range(NCORES)))
    out = res.results[0]["out"]
    return np.asarray([out[0, 0], out[0, 1]], dtype=np.float32)


# revision 27
# speedup vs baseline: 1.0835x; 1.0835x over previous
"""ArcFace-style sub-center loss (topk_masking) on 8 Trainium2 NeuronCores.

v4 strategy (class-parallel, 752 classes/core, pure-matmul device kernel):
  - Host pre-normalizes x and w rows (0.07% of model FLOPs), transposes
    both, and ships f16: xnT [512,1024], wnT [3,512,752] per core. The
    device does no norms and no input transposes; DMA is 3.3MB/core and
    the first cosine matmul issues at ~6us.
  - Per batch tile (128 rows): 24 f16 matmuls (k-major, 6 consecutive
    MMs share the stationary xnT block), psum chunks (512|240) merged
    over the 3 sub-centers by ACT copy + 2 DVE maxes into a contiguous
    [128,752] f32 cosine slab. Max8 writes the AG payload top-8 lanes
    directly; the label cosine is gathered by an (iota==label)*slab
    row-reduce; one ACT Exp pass with accum produces S_loc =
    sum(exp(30*cos)) in RAW exp space (args <= ~11, f32-safe), so no
    row-max bias pass and no AllReduce are needed anywhere.
  - ONE AllGather total ([1024,10] payload = top8 | cosl | S_loc).
    Collectives serialize on the gpsimd queue with ~10us dispatch + ~8us
    exec each, so fewer is strictly better; the CC ring arming happens
    during the matmul phase.
  - The AG result transpose ([8 ranks,1024,10] -> row-major) is done as
    ONE line-rate DMA into a [64,1280] staging tile plus 10 PE
    transposes, instead of ~8k 40-byte DMA descriptors.
  - Margin fixups in raw exp space; sine via Taylor 1 - c^2/2 - c^4/8
    (cosines here are < 0.4); ACT tables load exactly twice (Exp, Ln).
  - loss_row = ln(S + corr) - 30*phi_l; loss/prec cross-partition
    reduced by a ones-matmul; core 0 returns the [1,2] result.
"""

import math

import numpy as np

import concourse.bass as bass
import concourse.mybir as mybir
import concourse.tile as tile
from concourse import bacc
from concourse.bass import ds, ts
from concourse.bass_utils import run_bass_kernel_spmd
from concourse.masks import make_identity

F32 = mybir.dt.float32
F16 = mybir.dt.float16
I32 = mybir.dt.int32
AOP = mybir.AluOpType
AF = mybir.ActivationFunctionType
AX = mybir.AxisListType

B, NOUT, NCLASSES, CENTER, TOPK = 1024, 512, 5994, 3, 5
NCORES = 8
CPCW = 752                    # classes per core (core 7: 730 real + 22 pad)
NPAD = float(NCORES * CPCW - NCLASSES)  # 22 zero-weight pad columns
NBT = B // 128                # 8 batch tiles
KT = NOUT // 128              # 4 contraction chunks
N0, N1 = 512, CPCW - 512      # psum chunk widths (bank-aligned)
SCALE = 30.0
AGW = 10                      # AG payload floats/row: top8 | cosl | S_loc

M, SUB_M = 0.2, -0.06
COS_M, SIN_M = math.cos(M), math.sin(M)
SUB_COS_M, SUB_SIN_M = math.cos(SUB_M), math.sin(SUB_M)

_CACHE = {}


def _build():
    nc = bacc.Bacc("TRN2", target_bir_lowering=False, debug=False,
                   num_devices=NCORES)
    x_d = nc.dram_tensor("xnT", [NOUT, B], F16, kind="ExternalInput")
    w_d = nc.dram_tensor("wnT", [CENTER, NOUT, CPCW], F16,
                         kind="ExternalInput")
    lab_d = nc.dram_tensor("labels", [128, NBT], F32, kind="ExternalInput")
    out_d = nc.dram_tensor("out", [128, 2], F32, kind="ExternalOutput")

    with tile.TileContext(nc) as tc:
        with (
            tc.tile_pool(name="const", bufs=1) as constp,
            tc.tile_pool(name="big", bufs=1) as bigp,
            tc.tile_pool(name="slab", bufs=4) as slabp,
            tc.tile_pool(name="scr", bufs=2) as scrp,
            tc.tile_pool(name="gscr", bufs=2) as gscrp,
            tc.tile_pool(name="small", bufs=1) as smallp,
            tc.tile_pool(name="pay", bufs=NBT) as payp,
            tc.tile_pool(name="psA", bufs=4, space="PSUM") as psA,
            tc.tile_pool(name="dram", bufs=1, space="DRAM") as dramp,
        ):
            # ---- constants (gpsimd queue: consts, then ONLY the AG) ----
            iota_i = constp.tile([128, CPCW], I32, tag="iotai")
            nc.gpsimd.iota(iota_i[:], pattern=[[1, CPCW]], base=0,
                           channel_multiplier=0)
            identity = constp.tile([128, 128], F32, tag="ident")
            make_identity(nc, identity[:])
            ones = constp.tile([128, 1], F32, tag="ones")
            nc.vector.memset(ones[:], 1.0)
            iota_f = constp.tile([128, CPCW], F32, tag="iotaf")
            nc.vector.tensor_copy(iota_f[:], iota_i[:])
            labs = constp.tile([128, NBT], F32, tag="labs")
            nc.sync.dma_start(labs[:], lab_d[:])


            # ---- inputs: already normalized + transposed on host ----
            xnT = bigp.tile([128, KT, B], F16, tag="xnT")
            wnT = bigp.tile([128, CENTER, KT, CPCW], F16, tag="wnT")
            for k in range(KT):
                for a in range(CENTER):
                    nc.sync.dma_start(wnT[:, a, k, :],
                                      w_d[a, ds(k * 128, 128), :])
                nc.sync.dma_start(xnT[:, k, :], x_d[ds(k * 128, 128), :])

            # ---- per-batch-tile: cosine slab, top8, label gather, exp ----
            pays = [payp.tile([128, AGW], F32, tag="pay", name=f"pay{t}")
                    for t in range(NBT)]
            ag_ins = [dramp.tile([B // 2, AGW], F32, tag=f"agin{h}",
                                 name=f"agin{h}") for h in range(2)]

            # ---- two AllGathers: [bt 0-3] and [bt 4-7] ----
            # AG1's dispatch gap + mesh handshake overlap the matmul phase;
            # AG2 (second collective) begins ~1us after its trigger. Half-0
            # gather/merge/fixups run during AG2's flight.
            HB = NBT // 2
            ag_outs = [dramp.tile([NCORES, HB * 128, AGW], F32,
                                  tag=f"agout{h}", name=f"agout{h}")
                       for h in range(2)]
            stages = [smallp.tile([NCORES * HB, 128 * AGW], F32,
                                  tag=f"stage{h}", name=f"stage{h}")
                      for h in range(2)]
            gallTs = [smallp.tile([128, AGW * NCORES * HB], F32,
                                  tag=f"gallT{h}", name=f"gallT{h}")
                      for h in range(2)]
            g8 = smallp.tile([128, NBT * 8], F32, tag="g8")
            g3 = g8[:].rearrange("p (t k) -> p t k", k=8)
            CS = smallp.tile([128, 2 * NBT], F32, tag="CS")
            CS3 = CS[:].rearrange("p (j t) -> p j t", j=2)
            cosl = CS3[:, 0, :]
            Sg = CS3[:, 1, :]
            SC = smallp.tile([128, NBT], F32, tag="SC")

            def emit_ag(h):
                nc.gpsimd.collective_compute(
                    "AllGather", AOP.bypass,
                    replica_groups=[list(range(NCORES))],
                    ins=[ag_ins[h][:].opt()],
                    outs=[ag_outs[h][:].opt()])



            for bt in range(NBT):
                slab = slabp.tile([128, CPCW], F32, tag="slab")
                pas = [psA.tile([128, 2, N0], F32, tag="psA",
                                name=f"psA_{bt}_{a}") for a in range(CENTER)]
                for k in range(KT):
                    lhs = xnT[:, k, ts(bt, 128)]
                    for a in range(CENTER):
                        nc.tensor.matmul(pas[a][:, 0, :], lhs,
                                         wnT[:, a, k, 0:N0],
                                         start=(k == 0), stop=(k == KT - 1))
                        nc.tensor.matmul(pas[a][:, 1, 0:N1], lhs,
                                         wnT[:, a, k, N0:CPCW],
                                         start=(k == 0), stop=(k == KT - 1))
                pavs = [p[:].rearrange("p a b -> p (a b)")[:, 0:CPCW]
                        for p in pas]
                nc.scalar.copy(slab[:], pavs[0])
                nc.vector.tensor_tensor(slab[:], pavs[1], slab[:], op=AOP.max)
                nc.vector.tensor_tensor(slab[:], pavs[2], slab[:], op=AOP.max)
                nc.vector.max(pays[bt][:, 0:8], slab[:])
                gscr = gscrp.tile([128, CPCW], F32, tag="gscr")
                nc.vector.scalar_tensor_tensor(
                    out=gscr[:], in0=iota_f[:], scalar=labs[:, ds(bt, 1)],
                    in1=slab[:], op0=AOP.is_equal, op1=AOP.mult,
                    accum_out=pays[bt][:, 8:9])
                escr = scrp.tile([128, CPCW], F32, tag="scr750")
                nc.scalar.activation(escr[:], slab[:], AF.Exp, scale=SCALE,
                                     accum_out=pays[bt][:, 9:10])
                nc.sync.dma_start(
                    ag_ins[bt // HB][ts(bt % HB, 128), :], pays[bt][:])
                if bt == HB - 1:
                    emit_ag(0)
                elif bt == NBT - 1:
                    emit_ag(1)

            def emit_gather(h):
                agv = ag_outs[h][:].rearrange("c (t p) j -> (c t) (p j)",
                                              p=128)
                nc.sync.dma_start(stages[h][:], agv)
                gallT4 = gallTs[h][:].rearrange("p (j c t) -> p j c t",
                                                c=NCORES, t=HB)
                stg3 = stages[h][:].rearrange("q (p j) -> q p j", j=AGW)
                NQ = NCORES * HB
                for jj in range(AGW // 2):
                    pg = psA.tile([128, 2, N0], F32, tag="psA",
                                  name=f"psG{h}_{jj}")
                    for j2 in range(2):
                        j = 2 * jj + j2
                        nc.tensor.transpose(pg[:, 0, ds(j2 * NQ, NQ)],
                                            stg3[:, :, j],
                                            identity[0:NQ, 0:NQ])
                    nc.vector.tensor_copy(
                        gallTs[h][:, ds(jj * 2 * NQ, 2 * NQ)],
                        pg[:, 0, 0:2 * NQ])
                for t in range(HB):
                    bt = h * HB + t
                    nc.vector.max(g8[:, ts(bt, 8)], gallT4[:, 0:8, :, t])
                    nc.vector.tensor_reduce(CS3[:, :, bt],
                                            gallT4[:, 8:10, :, t],
                                            axis=AX.X, op=AOP.add)

            def t64(tag, half, w=8):
                return smallp.tile([128, HB * w], F32, tag=f"{tag}{half}",
                                   name=f"{tag}{half}")

            PHILS = []

            def emit_fix(half):
                th = ds(half * HB, HB)
                # A lanes per bt: 0:7 sub_phi args (6 top + label), 7 phi_l,
                # 8:14 raw top-6, 14 raw label cosine, 15 zero.
                A = t64("fxA", half, 16)
                E = t64("fxE", half, 16)
                S2 = t64("fxS2", half)
                Q4 = t64("fxQ4", half)
                SN = t64("fxSN", half)
                isin = t64("fxisin", half, 1)
                A4 = A[:].rearrange("p (t k) -> p t k", k=16)
                E4 = E[:].rearrange("p (t k) -> p t k", k=16)
                S23 = S2[:].rearrange("p (t k) -> p t k", k=8)[:, :, 0:7]
                Q43 = Q4[:].rearrange("p (t k) -> p t k", k=8)[:, :, 0:7]
                SN3 = SN[:].rearrange("p (t k) -> p t k", k=8)
                sn = SN3[:, :, 0:7]
                V = A4[:, :, 8:15]
                PHIL = A4[:, :, 7]
                PHILS.append(PHIL)
                nc.vector.memset(A4[:, :, 15], 0.0)
                nc.vector.tensor_copy(A4[:, :, 8:14], g3[:, th, 0:6])
                nc.vector.tensor_copy(A4[:, :, 14], cosl[:, th])
                # sine = 1 - c^2/2 - c^4/8  (|c| < 0.4 here)
                nc.vector.tensor_tensor(S23, V, V, op=AOP.mult)
                nc.vector.tensor_tensor(Q43, S23, S23, op=AOP.mult)
                nc.vector.tensor_scalar(sn, S23, -0.5, 1.0,
                                        op0=AOP.mult, op1=AOP.add)
                nc.vector.scalar_tensor_tensor(sn, Q43, -0.125, sn,
                                               op0=AOP.mult, op1=AOP.add)
                snl_m = SN3[:, :, 7]
                nc.vector.tensor_scalar_mul(snl_m, SN3[:, :, 6], SIN_M)
                nc.vector.scalar_tensor_tensor(PHIL, A4[:, :, 14], COS_M,
                                               snl_m,
                                               op0=AOP.mult, op1=AOP.subtract)
                nc.vector.tensor_scalar_mul(sn, sn, -SUB_SIN_M)
                nc.vector.scalar_tensor_tensor(A4[:, :, 0:7], V, SUB_COS_M,
                                               sn, op0=AOP.mult, op1=AOP.add)
                nc.scalar.activation(E[:], A[:], AF.Exp, scale=SCALE)
                nc.vector.tensor_tensor(E4[:, :, 0:7], E4[:, :, 0:7],
                                        E4[:, :, 8:15], op=AOP.subtract)
                isv = isin[:].rearrange("p (t k) -> p t k", k=1)
                nc.vector.tensor_tensor(isv[:, :, 0], A4[:, :, 14],
                                        g3[:, th, 5], op=AOP.is_ge)
                nc.vector.tensor_tensor(E4[:, :, 5:7], E4[:, :, 5:7],
                                        isv.to_broadcast([128, HB, 2]),
                                        op=AOP.mult)
                sumF = Q4[:].rearrange("p (t k) -> p t k", k=8)[:, :, 7]
                nc.vector.tensor_reduce(sumF, E4[:, :, 0:6], axis=AX.X,
                                        op=AOP.add)
                nc.vector.tensor_tensor(sumF, sumF, E4[:, :, 6],
                                        op=AOP.subtract)
                nc.vector.tensor_tensor(sumF, sumF, E4[:, :, 7], op=AOP.add)
                nc.vector.tensor_tensor(sumF, sumF, E4[:, :, 14],
                                        op=AOP.subtract)
                nc.vector.scalar_tensor_tensor(SC[:, th], sumF, -NPAD,
                                               Sg[:, th],
                                               op0=AOP.add, op1=AOP.add)

            emit_gather(0)
            emit_fix(0)
            emit_gather(1)
            emit_fix(1)

            # ---- loss / prec reduction ----
            # ln(SC) without the Ln table (the Exp table stays resident):
            # y0 from the f32 exponent bits + linear mantissa, then two
            # Newton steps y += SC*e^-y - 1.
            LN2 = math.log(2.0)
            lnS = smallp.tile([128, NBT], F32, tag="lnS")
            mant = smallp.tile([128, NBT], F32, tag="mant")
            ef = smallp.tile([128, NBT], F32, tag="ef")
            en = smallp.tile([128, NBT], F32, tag="en")
            v = smallp.tile([128, NBT], F32, tag="v")
            stacked = smallp.tile([128, 2], F32, tag="stacked")
            SCi = SC[:].bitcast(I32)
            nc.vector.tensor_scalar(mant[:].bitcast(I32), SCi,
                                    0x7FFFFF, 0x3F800000,
                                    op0=AOP.bitwise_and, op1=AOP.bitwise_or)
            nc.vector.tensor_scalar(ef[:].bitcast(I32), SCi, 23, None,
                                    op0=AOP.logical_shift_right)
            nc.vector.tensor_copy(ef[:], ef[:].bitcast(I32))
            # y0 = ef*ln2 + [(m-1)*0.7071 - 127*ln2]
            nc.vector.tensor_scalar(mant[:], mant[:], 0.7071,
                                    -0.7071 - 127.0 * LN2,
                                    op0=AOP.mult, op1=AOP.add)
            nc.vector.scalar_tensor_tensor(lnS[:], ef[:], LN2, mant[:],
                                           op0=AOP.mult, op1=AOP.add)
            for _ in range(1):
                nc.scalar.activation(en[:], lnS[:], AF.Exp, scale=-1.0)
                nc.vector.tensor_tensor(en[:], en[:], SC[:], op=AOP.mult)
                nc.vector.scalar_tensor_tensor(lnS[:], lnS[:], -1.0, en[:],
                                               op0=AOP.add, op1=AOP.add)
            # loss_row = (lnS - 30*phi_l)/B ; prec_row = (cosl>=t1)*100/B
            for h in range(2):
                th = ds(h * (NBT // 2), NBT // 2)
                nc.vector.scalar_tensor_tensor(lnS[:, th], PHILS[h], -SCALE,
                                               lnS[:, th],
                                               op0=AOP.mult, op1=AOP.add)
            nc.vector.tensor_scalar(mant[:], lnS[:], 1.0 / B, 0.0,
                                    op0=AOP.mult, op1=AOP.add,
                                    accum_out=stacked[:, 0:1])
            nc.vector.tensor_tensor(v[:], cosl, g3[:, :, 0], op=AOP.is_ge)
            nc.vector.tensor_scalar(ef[:], v[:], 100.0 / B, 0.0,
                                    op0=AOP.mult, op1=AOP.add,
                                    accum_out=stacked[:, 1:2])
            nc.sync.dma_start(out_d[:], stacked[:])

    nc.compile()
    return nc


def _in_maps(x, weight, label):
    x32 = np.asarray(x, dtype=np.float32)
    xn = x32 / np.maximum(np.linalg.norm(x32, axis=1, keepdims=True), 1e-12)
    xnT = np.ascontiguousarray(xn.T).astype(np.float16)   # [512, 1024]

    w32 = np.asarray(weight, dtype=np.float32)
    wn = w32 / np.maximum(np.linalg.norm(w32, axis=2, keepdims=True), 1e-12)
    wpad = np.zeros((CENTER, NCORES * CPCW, NOUT), np.float32)
    wpad[:, :NCLASSES] = wn
    lab = np.asarray(label).astype(np.int64)

    in_maps = []
    for m in range(NCORES):
        wslab = np.ascontiguousarray(
            wpad[:, m * CPCW:(m + 1) * CPCW].transpose(0, 2, 1)
        ).astype(np.float16)                               # [3, 512, 752]
        loc = lab - m * CPCW
        loc = np.where((loc >= 0) & (loc < CPCW), loc, -1)
        labs = np.ascontiguousarray(
            loc.reshape(NBT, 128).T.astype(np.float32))
        in_maps.append({"xnT": xnT, "wnT": wslab, "labels": labs})
    return in_maps


def kernel(x, weight, label):
    if "nc" not in _CACHE:
        _CACHE["nc"] = _build()
    nc = _CACHE["nc"]
    in_maps = _in_maps(x, weight, label)
    res = run_bass_kernel_spmd(nc, in_maps, core_ids=list(range(NCORES)))
    out = np.asarray(res.results[0]["out"], dtype=np.float32).sum(axis=0)
    return np.asarray([out[0], out[1]], dtype=np.float32)


# revision 29
# speedup vs baseline: 1.1650x; 1.0753x over previous
"""ArcFace-style sub-center loss (topk_masking) on 8 Trainium2 NeuronCores.

v4 strategy (class-parallel, 752 classes/core, pure-matmul device kernel):
  - Host pre-normalizes x and w rows (0.07% of model FLOPs), transposes
    both, and ships f16: xnT [512,1024], wnT [3,512,752] per core. The
    device does no norms and no input transposes; DMA is 3.3MB/core and
    the first cosine matmul issues at ~6us.
  - Per batch tile (128 rows): 24 f16 matmuls (k-major, 6 consecutive
    MMs share the stationary xnT block), psum chunks (512|240) merged
    over the 3 sub-centers by ACT copy + 2 DVE maxes into a contiguous
    [128,752] f32 cosine slab. Max8 writes the AG payload top-8 lanes
    directly; the label cosine is gathered by an (iota==label)*slab
    row-reduce; one ACT Exp pass with accum produces S_loc =
    sum(exp(30*cos)) in RAW exp space (args <= ~11, f32-safe), so no
    row-max bias pass and no AllReduce are needed anywhere.
  - ONE AllGather total ([1024,10] payload = top8 | cosl | S_loc).
    Collectives serialize on the gpsimd queue with ~10us dispatch + ~8us
    exec each, so fewer is strictly better; the CC ring arming happens
    during the matmul phase.
  - The AG result transpose ([8 ranks,1024,10] -> row-major) is done as
    ONE line-rate DMA into a [64,1280] staging tile plus 10 PE
    transposes, instead of ~8k 40-byte DMA descriptors.
  - Margin fixups in raw exp space; sine via Taylor 1 - c^2/2 - c^4/8
    (cosines here are < 0.4); ACT tables load exactly twice (Exp, Ln).
  - loss_row = ln(S + corr) - 30*phi_l; loss/prec cross-partition
    reduced by a ones-matmul; core 0 returns the [1,2] result.
"""

import math

import numpy as np

import concourse.bass as bass
import concourse.mybir as mybir
import concourse.tile as tile
from concourse import bacc
from concourse.bass import ds, ts
from concourse.bass_utils import run_bass_kernel_spmd
from concourse.masks import make_identity

F32 = mybir.dt.float32
F16 = mybir.dt.float16
I32 = mybir.dt.int32
AOP = mybir.AluOpType
AF = mybir.ActivationFunctionType
AX = mybir.AxisListType

B, NOUT, NCLASSES, CENTER, TOPK = 1024, 512, 5994, 3, 5
NCORES = 8
CPCW = 752                    # classes per core (core 7: 730 real + 22 pad)
NPAD = float(NCORES * CPCW - NCLASSES)  # 22 zero-weight pad columns
NBT = B // 128                # 8 batch tiles
KT = NOUT // 128              # 4 contraction chunks
N0, N1 = 512, CPCW - 512      # psum chunk widths (bank-aligned)
SCALE = 30.0
AGW = 10                      # AG payload floats/row: top8 | cosl | S_loc

M, SUB_M = 0.2, -0.06
COS_M, SIN_M = math.cos(M), math.sin(M)
SUB_COS_M, SUB_SIN_M = math.cos(SUB_M), math.sin(SUB_M)

_CACHE = {}


def _build():
    nc = bacc.Bacc("TRN2", target_bir_lowering=False, debug=False,
                   num_devices=NCORES)
    x_d = nc.dram_tensor("xnT", [NOUT, B], F16, kind="ExternalInput")
    w_d = nc.dram_tensor("wnT", [CENTER, NOUT, CPCW], F16,
                         kind="ExternalInput")
    lab_d = nc.dram_tensor("labels", [128, NBT], F32, kind="ExternalInput")
    out_d = nc.dram_tensor("out", [128, 2], F32, kind="ExternalOutput")

    with tile.TileContext(nc) as tc:
        with (
            tc.tile_pool(name="const", bufs=1) as constp,
            tc.tile_pool(name="big", bufs=1) as bigp,
            tc.tile_pool(name="slab", bufs=4) as slabp,
            tc.tile_pool(name="scr", bufs=2) as scrp,
            tc.tile_pool(name="gscr", bufs=2) as gscrp,
            tc.tile_pool(name="small", bufs=1) as smallp,
            tc.tile_pool(name="pay", bufs=NBT) as payp,
            tc.tile_pool(name="psA", bufs=4, space="PSUM") as psA,
            tc.tile_pool(name="dram", bufs=1, space="DRAM") as dramp,
        ):
            # ---- constants (gpsimd queue: consts, then ONLY the AG) ----
            iota_i = constp.tile([128, CPCW], I32, tag="iotai")
            nc.gpsimd.iota(iota_i[:], pattern=[[1, CPCW]], base=0,
                           channel_multiplier=0)
            identity = constp.tile([128, 128], F32, tag="ident")
            make_identity(nc, identity[:])
            ones = constp.tile([128, 1], F32, tag="ones")
            nc.vector.memset(ones[:], 1.0)
            iota_f = constp.tile([128, CPCW], F32, tag="iotaf")
            nc.vector.tensor_copy(iota_f[:], iota_i[:])
            labs = constp.tile([128, NBT], F32, tag="labs")
            nc.sync.dma_start(labs[:], lab_d[:])


            # ---- inputs: already normalized + transposed on host ----
            xnT = bigp.tile([128, KT, B], F16, tag="xnT")
            wnT = bigp.tile([128, CENTER, KT, CPCW], F16, tag="wnT")
            for k in range(KT):
                for a in range(CENTER):
                    nc.sync.dma_start(wnT[:, a, k, :],
                                      w_d[a, ds(k * 128, 128), :])
                nc.sync.dma_start(xnT[:, k, :], x_d[ds(k * 128, 128), :])

            # ---- per-batch-tile: cosine slab, top8, label gather, exp ----
            pays = [payp.tile([128, AGW], F32, tag="pay", name=f"pay{t}")
                    for t in range(NBT)]
            ag_ins = [dramp.tile([B // 2, AGW], F32, tag=f"agin{h}",
                                 name=f"agin{h}") for h in range(2)]

            # ---- two AllGathers: [bt 0-3] and [bt 4-7] ----
            # AG1's dispatch gap + mesh handshake overlap the matmul phase;
            # AG2 (second collective) begins ~1us after its trigger. Half-0
            # gather/merge/fixups run during AG2's flight.
            HB = NBT // 2
            ag_outs = [dramp.tile([NCORES, HB * 128, AGW], F32,
                                  tag=f"agout{h}", name=f"agout{h}")
                       for h in range(2)]
            stages = [smallp.tile([NCORES * HB, 128 * AGW], F32,
                                  tag=f"stage{h}", name=f"stage{h}")
                      for h in range(2)]
            gallTs = [smallp.tile([128, AGW * NCORES * HB], F32,
                                  tag=f"gallT{h}", name=f"gallT{h}")
                      for h in range(2)]
            g8 = smallp.tile([128, NBT * 8], F32, tag="g8")
            g3 = g8[:].rearrange("p (t k) -> p t k", k=8)
            CS = smallp.tile([128, 2 * NBT], F32, tag="CS")
            CS3 = CS[:].rearrange("p (j t) -> p j t", j=2)
            cosl = CS3[:, 0, :]
            Sg = CS3[:, 1, :]
            SC = smallp.tile([128, NBT], F32, tag="SC")

            def emit_ag(h):
                nc.gpsimd.collective_compute(
                    "AllGather", AOP.bypass,
                    replica_groups=[list(range(NCORES))],
                    ins=[ag_ins[h][:].opt()],
                    outs=[ag_outs[h][:].opt()])



            for bt in range(NBT):
                slab = slabp.tile([128, CPCW], F32, tag="slab")
                pas = [psA.tile([128, 2, N0], F32, tag="psA",
                                name=f"psA_{bt}_{a}") for a in range(CENTER)]
                for k in range(KT):
                    lhs = xnT[:, k, ts(bt, 128)]
                    for a in range(CENTER):
                        nc.tensor.matmul(pas[a][:, 0, :], lhs,
                                         wnT[:, a, k, 0:N0],
                                         start=(k == 0), stop=(k == KT - 1))
                        nc.tensor.matmul(pas[a][:, 1, 0:N1], lhs,
                                         wnT[:, a, k, N0:CPCW],
                                         start=(k == 0), stop=(k == KT - 1))
                pavs = [p[:].rearrange("p a b -> p (a b)")[:, 0:CPCW]
                        for p in pas]
                nc.scalar.copy(slab[:], pavs[0])
                nc.vector.tensor_tensor(slab[:], pavs[1], slab[:], op=AOP.max)
                nc.vector.tensor_tensor(slab[:], pavs[2], slab[:], op=AOP.max)
                nc.vector.max(pays[bt][:, 0:8], slab[:])
                gscr = gscrp.tile([128, CPCW], F32, tag="gscr")
                nc.vector.scalar_tensor_tensor(
                    out=gscr[:], in0=iota_f[:], scalar=labs[:, ds(bt, 1)],
                    in1=slab[:], op0=AOP.is_equal, op1=AOP.mult,
                    accum_out=pays[bt][:, 8:9])
                escr = scrp.tile([128, CPCW], F32, tag="scr750")
                nc.scalar.activation(escr[:], slab[:], AF.Exp, scale=SCALE,
                                     accum_out=pays[bt][:, 9:10])
                nc.sync.dma_start(
                    ag_ins[bt // HB][ts(bt % HB, 128), :], pays[bt][:])
                if bt == HB - 1:
                    emit_ag(0)
                elif bt == NBT - 1:
                    emit_ag(1)

            def emit_gather(h):
                agv = ag_outs[h][:].rearrange("c (t p) j -> (c t) (p j)",
                                              p=128)
                if h == 0:
                    nc.sync.dma_start(stages[h][:], agv)
                else:
                    nc.sync.dma_start(stages[h][0:16, :], agv[0:16, :])
                    nc.gpsimd.dma_start(stages[h][16:32, :], agv[16:32, :])
                gallT4 = gallTs[h][:].rearrange("p (j c t) -> p j c t",
                                                c=NCORES, t=HB)
                stg3 = stages[h][:].rearrange("q (p j) -> q p j", j=AGW)
                NQ = NCORES * HB
                for jj in range(AGW // 2):
                    pg = psA.tile([128, 2, N0], F32, tag="psA",
                                  name=f"psG{h}_{jj}")
                    for j2 in range(2):
                        j = 2 * jj + j2
                        nc.tensor.transpose(pg[:, 0, ds(j2 * NQ, NQ)],
                                            stg3[:, :, j],
                                            identity[0:NQ, 0:NQ])
                    nc.vector.tensor_copy(
                        gallTs[h][:, ds(jj * 2 * NQ, 2 * NQ)],
                        pg[:, 0, 0:2 * NQ])
                for t in range(HB):
                    bt = h * HB + t
                    nc.vector.max(g8[:, ts(bt, 8)], gallT4[:, 0:8, :, t])
                    nc.vector.tensor_reduce(CS3[:, :, bt],
                                            gallT4[:, 8:10, :, t],
                                            axis=AX.X, op=AOP.add)

            def t64(tag, half, w=8):
                return smallp.tile([128, HB * w], F32, tag=f"{tag}{half}",
                                   name=f"{tag}{half}")

            PHILS = []

            def emit_fix(half):
                th = ds(half * HB, HB)
                # A lanes per bt: 0:7 sub_phi args (6 top + label), 7 phi_l,
                # 8:14 raw top-6, 14 raw label cosine, 15 zero.
                A = t64("fxA", half, 16)
                E = t64("fxE", half, 16)
                S2 = t64("fxS2", half)
                Q4 = t64("fxQ4", half)
                SN = t64("fxSN", half)
                isin = t64("fxisin", half, 1)
                A4 = A[:].rearrange("p (t k) -> p t k", k=16)
                E4 = E[:].rearrange("p (t k) -> p t k", k=16)
                S23 = S2[:].rearrange("p (t k) -> p t k", k=8)[:, :, 0:7]
                Q43 = Q4[:].rearrange("p (t k) -> p t k", k=8)[:, :, 0:7]
                SN3 = SN[:].rearrange("p (t k) -> p t k", k=8)
                sn = SN3[:, :, 0:7]
                V = A4[:, :, 8:15]
                PHIL = A4[:, :, 7]
                PHILS.append(PHIL)
                nc.vector.memset(A4[:, :, 15], 0.0)
                nc.vector.tensor_copy(A4[:, :, 8:14], g3[:, th, 0:6])
                nc.vector.tensor_copy(A4[:, :, 14], cosl[:, th])
                # sine = 1 - c^2/2 - c^4/8  (|c| < 0.4 here)
                nc.vector.tensor_tensor(S23, V, V, op=AOP.mult)
                nc.vector.tensor_tensor(Q43, S23, S23, op=AOP.mult)
                nc.vector.tensor_scalar(sn, S23, -0.5, 1.0,
                                        op0=AOP.mult, op1=AOP.add)
                nc.vector.scalar_tensor_tensor(sn, Q43, -0.125, sn,
                                               op0=AOP.mult, op1=AOP.add)
                snl_m = SN3[:, :, 7]
                nc.vector.tensor_scalar_mul(snl_m, SN3[:, :, 6], SIN_M)
                nc.vector.scalar_tensor_tensor(PHIL, A4[:, :, 14], COS_M,
                                               snl_m,
                                               op0=AOP.mult, op1=AOP.subtract)
                nc.vector.tensor_scalar_mul(sn, sn, -SUB_SIN_M)
                nc.vector.scalar_tensor_tensor(A4[:, :, 0:7], V, SUB_COS_M,
                                               sn, op0=AOP.mult, op1=AOP.add)
                nc.scalar.activation(E[:], A[:], AF.Exp, scale=SCALE)
                nc.vector.tensor_tensor(E4[:, :, 0:7], E4[:, :, 0:7],
                                        E4[:, :, 8:15], op=AOP.subtract)
                isv = isin[:].rearrange("p (t k) -> p t k", k=1)
                nc.vector.tensor_tensor(isv[:, :, 0], A4[:, :, 14],
                                        g3[:, th, 5], op=AOP.is_ge)
                nc.vector.tensor_tensor(E4[:, :, 5:7], E4[:, :, 5:7],
                                        isv.to_broadcast([128, HB, 2]),
                                        op=AOP.mult)
                sumF = Q4[:].rearrange("p (t k) -> p t k", k=8)[:, :, 7]
                nc.vector.tensor_reduce(sumF, E4[:, :, 0:6], axis=AX.X,
                                        op=AOP.add)
                nc.vector.tensor_tensor(sumF, sumF, E4[:, :, 6],
                                        op=AOP.subtract)
                nc.vector.tensor_tensor(sumF, sumF, E4[:, :, 7], op=AOP.add)
                nc.vector.tensor_tensor(sumF, sumF, E4[:, :, 14],
                                        op=AOP.subtract)
                nc.vector.scalar_tensor_tensor(SC[:, th], sumF, -NPAD,
                                               Sg[:, th],
                                               op0=AOP.add, op1=AOP.add)

            emit_gather(0)
            emit_fix(0)
            emit_gather(1)
            emit_fix(1)

            # ---- loss / prec reduction ----
            # ln(SC) without the Ln table (the Exp table stays resident):
            # y0 from the f32 exponent bits + linear mantissa, then two
            # Newton steps y += SC*e^-y - 1.
            LN2 = math.log(2.0)
            lnS = smallp.tile([128, NBT], F32, tag="lnS")
            mant = smallp.tile([128, NBT], F32, tag="mant")
            ef = smallp.tile([128, NBT], F32, tag="ef")
            en = smallp.tile([128, NBT], F32, tag="en")
            v = smallp.tile([128, NBT], F32, tag="v")
            stacked = smallp.tile([128, 2], F32, tag="stacked")
            SCi = SC[:].bitcast(I32)
            nc.vector.tensor_scalar(mant[:].bitcast(I32), SCi,
                                    0x7FFFFF, 0x3F800000,
                                    op0=AOP.bitwise_and, op1=AOP.bitwise_or)
            nc.vector.tensor_scalar(ef[:].bitcast(I32), SCi, 23, None,
                                    op0=AOP.logical_shift_right)
            nc.vector.tensor_copy(ef[:], ef[:].bitcast(I32))
            # y0 = ef*ln2 + [(m-1)*0.7071 - 127*ln2]
            nc.vector.tensor_scalar(mant[:], mant[:], 0.7071,
                                    -0.7071 - 127.0 * LN2,
                                    op0=AOP.mult, op1=AOP.add)
            nc.vector.scalar_tensor_tensor(lnS[:], ef[:], LN2, mant[:],
                                           op0=AOP.mult, op1=AOP.add)
            for _ in range(1):
                nc.scalar.activation(en[:], lnS[:], AF.Exp, scale=-1.0)
                nc.vector.tensor_tensor(en[:], en[:], SC[:], op=AOP.mult)
                nc.vector.scalar_tensor_tensor(lnS[:], lnS[:], -1.0, en[:],
                                               op0=AOP.add, op1=AOP.add)
            # loss_row = (lnS - 30*phi_l)/B ; prec_row = (cosl>=t1)*100/B
            for h in range(2):
                th = ds(h * (NBT // 2), NBT // 2)
                nc.vector.scalar_tensor_tensor(lnS[:, th], PHILS[h], -SCALE,
                                               lnS[:, th],
                                               op0=AOP.mult, op1=AOP.add)
            nc.vector.tensor_scalar(mant[:], lnS[:], 1.0 / B, 0.0,
                                    op0=AOP.mult, op1=AOP.add,
                                    accum_out=stacked[:, 0:1])
            nc.vector.tensor_tensor(v[:], cosl, g3[:, :, 0], op=AOP.is_ge)
            nc.vector.tensor_scalar(ef[:], v[:], 100.0 / B, 0.0,
                                    op0=AOP.mult, op1=AOP.add,
                                    accum_out=stacked[:, 1:2])
            nc.sync.dma_start(out_d[:], stacked[:])

    nc.compile()
    return nc


def _in_maps(x, weight, label):
    x32 = np.asarray(x, dtype=np.float32)
    xn = x32 / np.maximum(np.linalg.norm(x32, axis=1, keepdims=True), 1e-12)
    xnT = np.ascontiguousarray(xn.T).astype(np.float16)   # [512, 1024]

    w32 = np.asarray(weight, dtype=np.float32)
    wn = w32 / np.maximum(np.linalg.norm(w32, axis=2, keepdims=True), 1e-12)
    wpad = np.zeros((CENTER, NCORES * CPCW, NOUT), np.float32)
    wpad[:, :NCLASSES] = wn
    lab = np.asarray(label).astype(np.int64)

    in_maps = []
    for m in range(NCORES):
        wslab = np.ascontiguousarray(
            wpad[:, m * CPCW:(m + 1) * CPCW].transpose(0, 2, 1)
        ).astype(np.float16)                               # [3, 512, 752]
        loc = lab - m * CPCW
        loc = np.where((loc >= 0) & (loc < CPCW), loc, -1)
        labs = np.ascontiguousarray(
            loc.reshape(NBT, 128).T.astype(np.float32))
        in_maps.append({"xnT": xnT, "wnT": wslab, "labels": labs})
    return in_maps


def kernel(x, weight, label):
    if "nc" not in _CACHE:
        _CACHE["nc"] = _build()
    nc = _CACHE["nc"]
    in_maps = _in_maps(x, weight, label)
    res = run_bass_kernel_spmd(nc, in_maps, core_ids=list(range(NCORES)))
    out = np.asarray(res.results[0]["out"], dtype=np.float32).sum(axis=0)
    return np.asarray([out[0], out[1]], dtype=np.float32)
